# revision 19
# baseline (speedup 1.0000x reference)
"""VQ codebook kernel for Trainium2 (8 NeuronCores, data-parallel over batch).

Pipeline per core (4 batches = 16384 tokens, chunks of 512 tokens):
  1. DMA z chunk (fp16 hi/lo pair, host-split) -> SBUF.
  2. PE: projection zp' = z @ (256*W).T into PSUM fp32 (multi-pass fp16).
  3. ACT: cast zp' -> fp16 (stationary for scores); optional DVE lo-part.
  4. PE: biased scores s[n,k] = zp'·(512*e_k) + C[k] into PSUM
     (C[k] = 256*512*(b·e_k - 0.5*||e_k||^2) via a rank-1 accumulate), so
     argmax_k s == argmin_k ||zp - e_k||^2 of the reference.
  5. DVE: max8 + find_index8 per 128-token tile -> argmax index per token.
  6. Index shuffle (DMA roundtrip through DRAM scratch) into the 16-partition
     wrapped layout ap_gather expects.
  7. GPSIMD ap_gather: z_q[d, n] = embT[d, idx_n] (exact fp32 emb values).
  8. DMA out z_q [d, hw] and indices.

Outputs exactly match the reference's (rep_z_q, min_indices) up to fp32
rounding of the argmin decision (rep_z_q == emb[min_indices] numerically).
"""

import numpy as np

B, C, H, W = 32, 256, 64, 64
D, K = 128, 512
NCORES = 8
B_PER = B // NCORES          # 4 batches per core
HW = H * W                   # 4096
CHUNK = 512                  # tokens per chunk
NCHUNK = B_PER * HW // CHUNK  # 32
NTOK = B_PER * HW            # 16384 tokens per core

SCALE_W = 256.0
SCALE_E = 512.0

# precision knobs: number of fp16 matmul passes
PROJ_PASSES = 3   # 1: zh*Wh | 2: +zl*Wh | 3: +zh*Wl
TIMING_HACK_SKIP_ISCR = False
SCORE_PASSES = 1  # 1: zph*Eh | 2: +zpl*Eh | 3: +zph*El

_cached = {}


def _build():
    import concourse.bass as bass
    from concourse import bacc
    import concourse.mybir as mybir
    import concourse.tile as tile

    f32 = mybir.dt.float32
    f16 = mybir.dt.float16
    i16 = mybir.dt.int16
    u32 = mybir.dt.uint32

    nc = bacc.Bacc(target_bir_lowering=False)

    zh_d = nc.dram_tensor("zh", [B_PER, C, HW], f16, kind="ExternalInput")
    if PROJ_PASSES >= 2:
        zl_d = nc.dram_tensor("zl", [B_PER, C, HW], f16, kind="ExternalInput")
    wh_d = nc.dram_tensor("wh", [2, 128, D], f16, kind="ExternalInput")
    if PROJ_PASSES >= 3:
        wl_d = nc.dram_tensor("wl", [2, 128, D], f16, kind="ExternalInput")
    eh_d = nc.dram_tensor("eh", [D, K], f16, kind="ExternalInput")
    if SCORE_PASSES >= 3:
        el_d = nc.dram_tensor("el", [D, K], f16, kind="ExternalInput")
    e32_d = nc.dram_tensor("e32", [D, K], f32, kind="ExternalInput")
    c16_d = nc.dram_tensor("c16", [1, K], f16, kind="ExternalInput")
    id128_d = nc.dram_tensor("id128", [128, 128], f32, kind="ExternalInput")
    i32f_d = nc.dram_tensor("i32f", [32, 32], f16, kind="ExternalInput")

    idx_out = nc.dram_tensor("idx", [NCHUNK, 128, 32], u32, kind="ExternalOutput")
    wdump = [nc.dram_tensor(f"wd{c}", [512], f16) for c in range(NCHUNK)]
    zq_out = nc.dram_tensor("zq", [B_PER, D, HW], f32, kind="ExternalOutput")


    with tile.TileContext(nc) as tc:
        with (
            tc.tile_pool(name="const", bufs=1) as cpool,
            tc.tile_pool(name="zin", bufs=6) as zpool,
            tc.tile_pool(name="work", bufs=4) as wpool,
            tc.tile_pool(name="small", bufs=12) as spool,
            tc.tile_pool(name="pzp", bufs=2, space="PSUM") as pzp_pool,
            tc.tile_pool(name="psc", bufs=4, space="PSUM") as psc_pool,
            tc.tile_pool(name="t1p", bufs=1, space="PSUM") as t1_pool,
            tc.tile_pool(name="twp", bufs=1, space="PSUM") as twp_pool,
        ):
            # constants, loaded once
            wh_sb = cpool.tile([128, 2, D], f16)
            nc.sync.dma_start(wh_sb[:], wh_d[:].rearrange("a p d -> p a d"))
            if PROJ_PASSES >= 3:
                wl_sb = cpool.tile([128, 2, D], f16)
                nc.sync.dma_start(wl_sb[:], wl_d[:].rearrange("a p d -> p a d"))
            eh_sb = cpool.tile([D, K], f16)
            nc.sync.dma_start(eh_sb[:], eh_d[:])
            if SCORE_PASSES >= 3:
                el_sb = cpool.tile([D, K], f16)
                nc.sync.dma_start(el_sb[:], el_d[:])
            e32_sb = cpool.tile([D, K], f32)
            nc.sync.dma_start(e32_sb[:], e32_d[:])
            c16_sb = cpool.tile([1, K], f16)
            nc.sync.dma_start(c16_sb[:], c16_d[:])
            ones_sb = cpool.tile([1, 128], f16)
            nc.vector.memset(ones_sb[:], 1.0)
            id128_sb = cpool.tile([128, 128], f32)
            nc.sync.dma_start(id128_sb[:], id128_d[:])
            i32f_sb = cpool.tile([32, 32], f16)
            nc.sync.dma_start(i32f_sb[:], i32f_d[:])

            # ---------------- phase 1: compute + indices to DRAM ----------
            # software-pipelined emission: proj for chunk ch+1 is emitted
            # before scores of chunk ch so the PE never waits on the cast.
            zph_of = {}
            tw_of = {}

            def emit_load_proj(ch):
                b = ch // (HW // CHUNK)
                col = (ch % (HW // CHUNK)) * CHUNK
                zh_sb = zpool.tile([128, 2, CHUNK], f16, tag="zh")
                nc.sync.dma_start(
                    zh_sb[:],
                    zh_d[b, :, col:col + CHUNK].rearrange("(a p) n -> p a n", p=128),
                )
                if PROJ_PASSES >= 2:
                    zl_sb = zpool.tile([128, 2, CHUNK], f16, tag="zl")
                    nc.sync.dma_start(
                        zl_sb[:],
                        zl_d[b, :, col:col + CHUNK].rearrange("(a p) n -> p a n", p=128),
                    )
                pzp = pzp_pool.tile([128, CHUNK], f32, tag="pzp")
                n_mm = 2 * PROJ_PASSES
                i_mm = 0
                for a in range(2):
                    nc.tensor.matmul(pzp[:], wh_sb[:, a, :], zh_sb[:, a, :],
                                     start=(i_mm == 0), stop=(i_mm == n_mm - 1))
                    i_mm += 1
                if PROJ_PASSES >= 2:
                    for a in range(2):
                        nc.tensor.matmul(pzp[:], wh_sb[:, a, :], zl_sb[:, a, :],
                                         start=False, stop=(i_mm == n_mm - 1))
                        i_mm += 1
                if PROJ_PASSES >= 3:
                    for a in range(2):
                        nc.tensor.matmul(pzp[:], wl_sb[:, a, :], zh_sb[:, a, :],
                                         start=False, stop=(i_mm == n_mm - 1))
                        i_mm += 1
                zph = wpool.tile([128, CHUNK], f16, tag="zph")
                nc.scalar.activation(zph[:], pzp[:],
                                     mybir.ActivationFunctionType.Copy)
                if SCORE_PASSES >= 2:
                    zpl = wpool.tile([128, CHUNK], f16, tag="zpl")
                    nc.vector.tensor_tensor(out=zpl[:], in0=pzp[:], in1=zph[:],
                                            op=mybir.AluOpType.subtract)
                else:
                    zpl = None
                zph_of[ch] = (zph, zpl)

            def emit_scores(ch):
                zph, zpl = zph_of.pop(ch)
                mx8 = spool.tile([128, 32], f32, tag="mx8")
                mi8 = spool.tile([128, 32], u32, tag="mi8")
                for t in range(4):
                    psc = psc_pool.tile([128, K], f32, tag="psc")
                    tok = slice(128 * t, 128 * (t + 1))
                    nc.tensor.matmul(psc[:], zph[:, tok], eh_sb[:],
                                     start=True, stop=False)
                    if SCORE_PASSES >= 2:
                        nc.tensor.matmul(psc[:], zpl[:, tok], eh_sb[:],
                                         start=False, stop=False)
                    if SCORE_PASSES >= 3:
                        nc.tensor.matmul(psc[:], zph[:, tok], el_sb[:],
                                         start=False, stop=False)
                    nc.tensor.matmul(psc[:], ones_sb[:], c16_sb[:],
                                     start=False, stop=True)
                    nc.vector.max(out=mx8[:, 8 * t:8 * t + 8], in_=psc[:])
                    nc.vector.max_index(out=mi8[:, 8 * t:8 * t + 8],
                                        in_max=mx8[:, 8 * t:8 * t + 8],
                                        in_values=psc[:])

                nc.gpsimd.dma_start(idx_out[ch], mi8[:])

                # on-chip wrapped-index build (no DRAM roundtrip):
                # mi8 -> fp32 -> PE transpose -> fp16 -> strided SBUF DMA
                # -> replicate via identity matmul -> int16 tw for ap_gather
                mi8f = spool.tile([128, 32], f32, tag="mi8f")
                nc.vector.tensor_copy(mi8f[:], mi8[:])
                t1 = t1_pool.tile([32, 128], f32, tag="t1")
                nc.tensor.transpose(t1[:], mi8f[:], id128_sb[:])
                t1s = spool.tile([32, 128], f16, tag="t1s")
                nc.scalar.activation(t1s[:], t1[:],
                                     mybir.ActivationFunctionType.Copy)
                # W[m=8q+s, r] = t1s[8q, 16s+r] = idx(token 16m+r)
                # (via DRAM hop: fat dump of rows {8q}, scrambled read-back)
                dump_src = t1s[:].rearrange("(q e) f -> q e f", e=8)[:, 0, :]
                nc.gpsimd.dma_start(
                    wdump[ch][:].rearrange("(q f) -> q f", q=4), dump_src)
                wt = spool.tile([32, 16], f16, tag="wt")
                w_src = wdump[ch][:].rearrange("(q s r) -> q s r", q=4, s=8)
                nc.gpsimd.dma_start(wt[:], w_src)
                # replicate columns 8x: w128[m, 16g+r] = W[m, r]
                w128 = spool.tile([32, 128], f16, tag="w128")
                nc.vector.tensor_copy(
                    w128[:].rearrange("p (g r) -> p g r", r=16),
                    wt[:].unsqueeze(1).broadcast_to([32, 8, 16]))
                # tw[p, m] = sum_c w128[c, p] * I[c, m] = W[m, p%16]
                twp = twp_pool.tile([128, 32], f32, tag="twp")
                nc.tensor.matmul(twp[:], w128[:], i32f_sb[:],
                                 start=True, stop=True)
                tw = spool.tile([128, 32], i16, tag="tw")
                nc.vector.tensor_copy(tw[:], twp[:])
                tw_of[ch] = tw

            def emit_gather(ch):
                tw = tw_of.pop(ch)
                zq_sb = wpool.tile([128, CHUNK], f32, tag="zq")
                nc.gpsimd.ap_gather(zq_sb[:], e32_sb[:], tw[:],
                                    channels=128, num_elems=K, d=1,
                                    num_idxs=CHUNK)
                b = ch // (HW // CHUNK)
                col = (ch % (HW // CHUNK)) * CHUNK
                nc.sync.dma_start(zq_out[b, :, col:col + CHUNK], zq_sb[:])

            emit_load_proj(0)
            for ch in range(NCHUNK):
                if ch + 1 < NCHUNK:
                    emit_load_proj(ch + 1)
                emit_scores(ch)
                if ch >= 1:
                    emit_gather(ch - 1)
            emit_gather(NCHUNK - 1)


    nc.compile()
    return nc


def kernel(z, proj_w, proj_b, emb):
    from concourse.bass_utils import run_bass_kernel_spmd

    z = np.asarray(z, dtype=np.float32)
    proj_w = np.asarray(proj_w, dtype=np.float32)
    proj_b = np.asarray(proj_b, dtype=np.float32)
    emb = np.asarray(emb, dtype=np.float32)

    # host-side constant prep (fp64 where it matters)
    Ws = (proj_w.astype(np.float64) * SCALE_W).T            # [C, D]
    wh = Ws.astype(np.float16)
    wl = (Ws - wh.astype(np.float64)).astype(np.float16)
    wh_t = np.ascontiguousarray(wh.reshape(2, 128, D))
    wl_t = np.ascontiguousarray(wl.reshape(2, 128, D))

    Es = (emb.astype(np.float64) * SCALE_E).T               # [D, K]
    eh = Es.astype(np.float16)
    el = (Es - eh.astype(np.float64)).astype(np.float16)
    e32 = np.ascontiguousarray(emb.T.astype(np.float32))    # [D, K] exact

    e64 = emb.astype(np.float64)
    Cbias = SCALE_W * SCALE_E * (e64 @ proj_b.astype(np.float64)
                                 - 0.5 * np.sum(e64 * e64, axis=1))
    c16 = Cbias.astype(np.float16).reshape(1, K)

    id128 = np.eye(128, dtype=np.float32)
    i32f = np.eye(32, dtype=np.float16)

    zr = z.reshape(NCORES, B_PER, C, HW)
    zh = zr.astype(np.float16)
    zl = (zr - zh.astype(np.float32)).astype(np.float16)

    key = (PROJ_PASSES, SCORE_PASSES)
    if key not in _cached:
        _cached[key] = _build()
    nc = _cached[key]

    in_maps = []
    for c in range(NCORES):
        m = {
            "zh": zh[c], "wh": wh_t, "eh": eh, "e32": e32, "c16": c16,
            "id128": id128, "i32f": i32f,
        }
        if PROJ_PASSES >= 2:
            m["zl"] = zl[c]
        if PROJ_PASSES >= 3:
            m["wl"] = wl_t
        if SCORE_PASSES >= 3:
            m["el"] = el
        in_maps.append(m)

    kernel.last_in_maps = in_maps
    res = run_bass_kernel_spmd(nc, in_maps, core_ids=list(range(NCORES)))
    kernel.last_result = res

    zq = np.stack([res.results[c]["zq"] for c in range(NCORES)])
    zq = zq.reshape(B, D, H, W)
    idxs = []
    for c in range(NCORES):
        a = res.results[c]["idx"]                      # [NCHUNK, 128, 32] u16
        a = a[:, :, 0::8]                              # [NCHUNK, 128, 4] tile cols
        idxs.append(a.transpose(0, 2, 1).reshape(-1))  # token = 128*t + p
    idx = np.concatenate(idxs).astype(np.int32)
    return zq, idx


# revision 20
# speedup vs baseline: 1.2359x; 1.2359x over previous
"""VQ codebook kernel for Trainium2 (8 NeuronCores, data-parallel over batch).

Pipeline per core (4 batches = 16384 tokens, chunks of 512 tokens):
  1. DMA z chunk (fp16 hi/lo pair, host-split) -> SBUF.
  2. PE: projection zp' = z @ (256*W).T into PSUM fp32 (multi-pass fp16).
  3. ACT: cast zp' -> fp16 (stationary for scores); optional DVE lo-part.
  4. PE: biased scores s[n,k] = zp'·(512*e_k) + C[k] into PSUM
     (C[k] = 256*512*(b·e_k - 0.5*||e_k||^2) via a rank-1 accumulate), so
     argmax_k s == argmin_k ||zp - e_k||^2 of the reference.
  5. DVE: max8 + find_index8 per 128-token tile -> argmax index per token.
  6. Index shuffle (DMA roundtrip through DRAM scratch) into the 16-partition
     wrapped layout ap_gather expects.
  7. GPSIMD ap_gather: z_q[d, n] = embT[d, idx_n] (exact fp32 emb values).
  8. DMA out z_q [d, hw] and indices.

Outputs exactly match the reference's (rep_z_q, min_indices) up to fp32
rounding of the argmin decision (rep_z_q == emb[min_indices] numerically).
"""

import numpy as np

B, C, H, W = 32, 256, 64, 64
D, K = 128, 512
NCORES = 8
B_PER = B // NCORES          # 4 batches per core
HW = H * W                   # 4096
CHUNK = 512                  # tokens per chunk
NCHUNK = B_PER * HW // CHUNK  # 32
NTOK = B_PER * HW            # 16384 tokens per core

SCALE_W = 256.0
SCALE_E = 512.0

# precision knobs: number of fp16 matmul passes
PROJ_PASSES = 3   # 1: zh*Wh | 2: +zl*Wh | 3: +zh*Wl
TIMING_HACK_SKIP_ISCR = False
SCORE_PASSES = 1  # 1: zph*Eh | 2: +zpl*Eh | 3: +zph*El

_cached = {}


def _build():
    import concourse.bass as bass
    from concourse import bacc
    import concourse.mybir as mybir
    import concourse.tile as tile

    f32 = mybir.dt.float32
    f16 = mybir.dt.float16
    i16 = mybir.dt.int16
    u32 = mybir.dt.uint32

    nc = bacc.Bacc(target_bir_lowering=False)

    zh_d = nc.dram_tensor("zh", [B_PER, C, HW], f16, kind="ExternalInput")
    if PROJ_PASSES >= 2:
        zl_d = nc.dram_tensor("zl", [B_PER, C, HW], f16, kind="ExternalInput")
    wh_d = nc.dram_tensor("wh", [2, 128, D], f16, kind="ExternalInput")
    if PROJ_PASSES >= 3:
        wl_d = nc.dram_tensor("wl", [2, 128, D], f16, kind="ExternalInput")
    eh_d = nc.dram_tensor("eh", [D, K], f16, kind="ExternalInput")
    if SCORE_PASSES >= 3:
        el_d = nc.dram_tensor("el", [D, K], f16, kind="ExternalInput")
    e32_d = nc.dram_tensor("e32", [D, K], f32, kind="ExternalInput")
    c16_d = nc.dram_tensor("c16", [1, K], f16, kind="ExternalInput")
    id128_d = nc.dram_tensor("id128", [128, 128], f32, kind="ExternalInput")
    i32f_d = nc.dram_tensor("i32f", [32, 32], f16, kind="ExternalInput")

    idx_out = nc.dram_tensor("idx", [NCHUNK, 128, 32], u32, kind="ExternalOutput")
    wdump = [nc.dram_tensor(f"wd{c}", [512], f16) for c in range(NCHUNK)]
    zq_out = nc.dram_tensor("zq", [B_PER, D, HW], f32, kind="ExternalOutput")


    with tile.TileContext(nc) as tc:
        with (
            tc.tile_pool(name="const", bufs=1) as cpool,
            tc.tile_pool(name="zin", bufs=6) as zpool,
            tc.tile_pool(name="work", bufs=4) as wpool,
            tc.tile_pool(name="small", bufs=12) as spool,
            tc.tile_pool(name="pzp", bufs=2, space="PSUM") as pzp_pool,
            tc.tile_pool(name="psc", bufs=4, space="PSUM") as psc_pool,
            tc.tile_pool(name="t1p", bufs=1, space="PSUM") as t1_pool,
            tc.tile_pool(name="twp", bufs=1, space="PSUM") as twp_pool,
        ):
            # constants, loaded once
            wh_sb = cpool.tile([128, 2, D], f16)
            nc.sync.dma_start(wh_sb[:], wh_d[:].rearrange("a p d -> p a d"))
            if PROJ_PASSES >= 3:
                wl_sb = cpool.tile([128, 2, D], f16)
                nc.sync.dma_start(wl_sb[:], wl_d[:].rearrange("a p d -> p a d"))
            eh_sb = cpool.tile([D, K], f16)
            nc.sync.dma_start(eh_sb[:], eh_d[:])
            if SCORE_PASSES >= 3:
                el_sb = cpool.tile([D, K], f16)
                nc.sync.dma_start(el_sb[:], el_d[:])
            e32_sb = cpool.tile([D, K], f32)
            nc.sync.dma_start(e32_sb[:], e32_d[:])
            c16_sb = cpool.tile([1, K], f16)
            nc.sync.dma_start(c16_sb[:], c16_d[:])
            ones_sb = cpool.tile([1, 128], f16)
            nc.vector.memset(ones_sb[:], 1.0)
            id128_sb = cpool.tile([128, 128], f32)
            nc.sync.dma_start(id128_sb[:], id128_d[:])
            i32f_sb = cpool.tile([32, 32], f16)
            nc.sync.dma_start(i32f_sb[:], i32f_d[:])

            # ---------------- phase 1: compute + indices to DRAM ----------
            # software-pipelined emission: proj for chunk ch+1 is emitted
            # before scores of chunk ch so the PE never waits on the cast.
            zph_of = {}
            tw_of = {}

            def emit_load_proj(ch):
                b = ch // (HW // CHUNK)
                col = (ch % (HW // CHUNK)) * CHUNK
                zh_sb = zpool.tile([128, 2, CHUNK], f16, tag="zh")
                nc.sync.dma_start(
                    zh_sb[:],
                    zh_d[b, :, col:col + CHUNK].rearrange("(a p) n -> p a n", p=128),
                )
                if PROJ_PASSES >= 2:
                    zl_sb = zpool.tile([128, 2, CHUNK], f16, tag="zl")
                    nc.sync.dma_start(
                        zl_sb[:],
                        zl_d[b, :, col:col + CHUNK].rearrange("(a p) n -> p a n", p=128),
                    )
                pzp = pzp_pool.tile([128, CHUNK], f32, tag="pzp")
                n_mm = 2 * PROJ_PASSES
                i_mm = 0
                for a in range(2):
                    nc.tensor.matmul(pzp[:], wh_sb[:, a, :], zh_sb[:, a, :],
                                     start=(i_mm == 0), stop=(i_mm == n_mm - 1))
                    i_mm += 1
                if PROJ_PASSES >= 2:
                    for a in range(2):
                        nc.tensor.matmul(pzp[:], wh_sb[:, a, :], zl_sb[:, a, :],
                                         start=False, stop=(i_mm == n_mm - 1))
                        i_mm += 1
                if PROJ_PASSES >= 3:
                    for a in range(2):
                        nc.tensor.matmul(pzp[:], wl_sb[:, a, :], zh_sb[:, a, :],
                                         start=False, stop=(i_mm == n_mm - 1))
                        i_mm += 1
                zph = wpool.tile([128, CHUNK], f16, tag="zph")
                nc.scalar.activation(zph[:], pzp[:],
                                     mybir.ActivationFunctionType.Copy)
                if SCORE_PASSES >= 2:
                    zpl = wpool.tile([128, CHUNK], f16, tag="zpl")
                    nc.vector.tensor_tensor(out=zpl[:], in0=pzp[:], in1=zph[:],
                                            op=mybir.AluOpType.subtract)
                else:
                    zpl = None
                zph_of[ch] = (zph, zpl)

            def emit_scores(ch):
                zph, zpl = zph_of.pop(ch)
                mx8 = spool.tile([128, 32], f32, tag="mx8")
                mi8 = spool.tile([128, 32], u32, tag="mi8")
                for t in range(4):
                    psc = psc_pool.tile([128, K], f32, tag="psc")
                    tok = slice(128 * t, 128 * (t + 1))
                    nc.tensor.matmul(psc[:], zph[:, tok], eh_sb[:],
                                     start=True, stop=False)
                    if SCORE_PASSES >= 2:
                        nc.tensor.matmul(psc[:], zpl[:, tok], eh_sb[:],
                                         start=False, stop=False)
                    if SCORE_PASSES >= 3:
                        nc.tensor.matmul(psc[:], zph[:, tok], el_sb[:],
                                         start=False, stop=False)
                    nc.tensor.matmul(psc[:], ones_sb[:], c16_sb[:],
                                     start=False, stop=True)
                    nc.vector.max(out=mx8[:, 8 * t:8 * t + 8], in_=psc[:])
                    nc.vector.max_index(out=mi8[:, 8 * t:8 * t + 8],
                                        in_max=mx8[:, 8 * t:8 * t + 8],
                                        in_values=psc[:])

                nc.gpsimd.dma_start(idx_out[ch], mi8[:])

                # on-chip wrapped-index build (no DRAM roundtrip):
                # mi8 -> fp32 -> PE transpose -> fp16 -> strided SBUF DMA
                # -> replicate via identity matmul -> int16 tw for ap_gather
                mi8f = spool.tile([128, 32], f32, tag="mi8f")
                nc.vector.tensor_copy(mi8f[:], mi8[:])
                t1 = t1_pool.tile([32, 128], f32, tag="t1")
                nc.tensor.transpose(t1[:], mi8f[:], id128_sb[:])
                t1s = spool.tile([32, 128], f16, tag="t1s")
                nc.scalar.activation(t1s[:], t1[:],
                                     mybir.ActivationFunctionType.Copy)
                # W[m=8q+s, r] = t1s[8q, 16s+r] = idx(token 16m+r)
                # (via DRAM hop: fat dump of rows {8q}, scrambled read-back)
                dump_src = t1s[:].rearrange("(q e) f -> q e f", e=8)[:, 0, :]
                nc.scalar.dma_start(
                    wdump[ch][:].rearrange("(q f) -> q f", q=4), dump_src)
                wt = spool.tile([32, 16], f16, tag="wt")
                w_src = wdump[ch][:].rearrange("(q s r) -> q s r", q=4, s=8)
                nc.scalar.dma_start(wt[:], w_src)
                # replicate columns 8x: w128[m, 16g+r] = W[m, r]
                w128 = spool.tile([32, 128], f16, tag="w128")
                nc.vector.tensor_copy(
                    w128[:].rearrange("p (g r) -> p g r", r=16),
                    wt[:].unsqueeze(1).broadcast_to([32, 8, 16]))
                # tw[p, m] = sum_c w128[c, p] * I[c, m] = W[m, p%16]
                twp = twp_pool.tile([128, 32], f32, tag="twp")
                nc.tensor.matmul(twp[:], w128[:], i32f_sb[:],
                                 start=True, stop=True)
                tw = spool.tile([128, 32], i16, tag="tw")
                nc.vector.tensor_copy(tw[:], twp[:])
                tw_of[ch] = tw

            def emit_gather(ch):
                tw = tw_of.pop(ch)
                zq_sb = wpool.tile([128, CHUNK], f32, tag="zq")
                nc.gpsimd.ap_gather(zq_sb[:], e32_sb[:], tw[:],
                                    channels=128, num_elems=K, d=1,
                                    num_idxs=CHUNK)
                b = ch // (HW // CHUNK)
                col = (ch % (HW // CHUNK)) * CHUNK
                nc.sync.dma_start(zq_out[b, :, col:col + CHUNK], zq_sb[:])

            emit_load_proj(0)
            for ch in range(NCHUNK):
                if ch + 1 < NCHUNK:
                    emit_load_proj(ch + 1)
                emit_scores(ch)
                if ch >= 2:
                    emit_gather(ch - 2)
            emit_gather(NCHUNK - 2)
            emit_gather(NCHUNK - 1)


    nc.compile()
    return nc


def kernel(z, proj_w, proj_b, emb):
    from concourse.bass_utils import run_bass_kernel_spmd

    z = np.asarray(z, dtype=np.float32)
    proj_w = np.asarray(proj_w, dtype=np.float32)
    proj_b = np.asarray(proj_b, dtype=np.float32)
    emb = np.asarray(emb, dtype=np.float32)

    # host-side constant prep (fp64 where it matters)
    Ws = (proj_w.astype(np.float64) * SCALE_W).T            # [C, D]
    wh = Ws.astype(np.float16)
    wl = (Ws - wh.astype(np.float64)).astype(np.float16)
    wh_t = np.ascontiguousarray(wh.reshape(2, 128, D))
    wl_t = np.ascontiguousarray(wl.reshape(2, 128, D))

    Es = (emb.astype(np.float64) * SCALE_E).T               # [D, K]
    eh = Es.astype(np.float16)
    el = (Es - eh.astype(np.float64)).astype(np.float16)
    e32 = np.ascontiguousarray(emb.T.astype(np.float32))    # [D, K] exact

    e64 = emb.astype(np.float64)
    Cbias = SCALE_W * SCALE_E * (e64 @ proj_b.astype(np.float64)
                                 - 0.5 * np.sum(e64 * e64, axis=1))
    c16 = Cbias.astype(np.float16).reshape(1, K)

    id128 = np.eye(128, dtype=np.float32)
    i32f = np.eye(32, dtype=np.float16)

    zr = z.reshape(NCORES, B_PER, C, HW)
    zh = zr.astype(np.float16)
    zl = (zr - zh.astype(np.float32)).astype(np.float16)

    key = (PROJ_PASSES, SCORE_PASSES)
    if key not in _cached:
        _cached[key] = _build()
    nc = _cached[key]

    in_maps = []
    for c in range(NCORES):
        m = {
            "zh": zh[c], "wh": wh_t, "eh": eh, "e32": e32, "c16": c16,
            "id128": id128, "i32f": i32f,
        }
        if PROJ_PASSES >= 2:
            m["zl"] = zl[c]
        if PROJ_PASSES >= 3:
            m["wl"] = wl_t
        if SCORE_PASSES >= 3:
            m["el"] = el
        in_maps.append(m)

    kernel.last_in_maps = in_maps
    res = run_bass_kernel_spmd(nc, in_maps, core_ids=list(range(NCORES)))
    kernel.last_result = res

    zq = np.stack([res.results[c]["zq"] for c in range(NCORES)])
    zq = zq.reshape(B, D, H, W)
    idxs = []
    for c in range(NCORES):
        a = res.results[c]["idx"]                      # [NCHUNK, 128, 32] u16
        a = a[:, :, 0::8]                              # [NCHUNK, 128, 4] tile cols
        idxs.append(a.transpose(0, 2, 1).reshape(-1))  # token = 128*t + p
    idx = np.concatenate(idxs).astype(np.int32)
    return zq, idx
